# revision 8
# baseline (speedup 1.0000x reference)
"""Trainium2 Bass kernel for nn_FEELModel (TreeLSTM + triplet embedding model).

Strategy:
- Data-parallel over batch B=512 across 8 NeuronCores (64 rows/core); embedding
  table and weights replicated per core.
- Embedding rows are fetched with dma_gather (SWDGE custom gather, one op per
  ~768 rows). The int16 index range is handled by a parity split: emb is viewed
  as [V/2, 2, D] pairs and even/odd tokens are gathered in separate calls whose
  pair index fits in int16.
- Mean-pooling runs on the PE: each gathered 128-row slab is the stationary
  operand; a host-built membership matrix (weight 1/L at [position, group]) is
  the moving operand, accumulating pooled vectors directly TRANSPOSED
  (feature-on-partition) in PSUM. Sequence pools use a batch-windowed
  membership (position->batch mapping is near-linear) to keep N small.
- TreeLSTM/similarity GEMMs use the weights directly as lhsT (natural layout).
- Triplet dots: elementwise ops + ones-column matmul partition reduction.
"""
import sys

if "/opt/trn_rl_repo" not in sys.path:
    sys.path.insert(0, "/opt/trn_rl_repo")

from contextlib import ExitStack

import numpy as np

import concourse.bass as bass
import concourse.bacc as bacc
import concourse.mybir as mybir
import concourse.tile as tile
from concourse.bass_utils import run_bass_kernel_spmd

F32 = mybir.dt.float32
I16 = mybir.dt.int16
AF = mybir.ActivationFunctionType
ALU = mybir.AluOpType

# Full-size problem config (hardcoded; harness contract).
B, NC_CORES, L, LQ, V, D, M, H, O = 512, 8, 64, 128, 50000, 512, 512, 256, 30
SPC = 6  # gather slabs (128 rows each) per dma_gather call

ATTR_KEYS = ["q_v", "q_a0", "n_a0", "q_a1", "n_a1", "q_a2", "n_a2"]
SEQ_KEYS = ["query", "pos", "neg"]


def _cap(n):
    """Per-parity index capacity: n/2 + 8 sigma margin, 128-aligned, <= ceil(n)."""
    sigma = int(np.sqrt(n) / 2)
    c = n // 2 + max(128, 8 * sigma)
    return min(((c + 127) // 128) * 128, ((n + 127) // 128) * 128)


def _seq_window(Bc, LQ):
    span = max(1, 256 // LQ)  # batch rows spanned by one 128-position slab
    return span, min(Bc, 2 * span + 12)


def _seq_base(s, span, W, Bc):
    return int(np.clip(span * s - (W - span) // 2, 0, Bc - W))


def build_program(Bc, L, LQ, V, D, M, H, O):
    DC = D // 128
    MC = M // 128
    HC = H // 128
    NPT = 4 * Bc          # pooled cols per tree (4b+node layout)
    LB = 3 * Bc
    PS_T = 256            # per-tree column stride in f psum
    CAP_A = _cap(Bc * L)
    CAP_S = _cap(Bc * LQ)
    SL_A = CAP_A // 128
    SL_S = CAP_S // 128
    span, WB = _seq_window(Bc, LQ)
    assert NPT <= 256 and 4 * WB <= NPT

    nc = bacc.Bacc("TRN2", target_bir_lowering=False, debug=False)

    emb_d = nc.dram_tensor("emb", (V, D), F32, kind="ExternalInput")
    idx_d = nc.dram_tensor("idx", (128, (3 * SL_S + 7 * SL_A) * 2 * 8), I16, kind="ExternalInput")
    memb_s_d = nc.dram_tensor("memb_s", (128, 3 * 2 * SL_S, 4 * WB), F32, kind="ExternalInput")
    memb_a_d = nc.dram_tensor("memb_a", (128, 7 * 2 * SL_A, Bc), F32, kind="ExternalInput")
    Wioux_d = nc.dram_tensor("Wioux", (D, 3 * M), F32, kind="ExternalInput")
    Wiouh_d = nc.dram_tensor("Wiouh", (M, 3 * M), F32, kind="ExternalInput")
    Wfx_d = nc.dram_tensor("Wfx", (D, M), F32, kind="ExternalInput")
    Wfh_d = nc.dram_tensor("Wfh", (M, M), F32, kind="ExternalInput")
    Wwh_d = nc.dram_tensor("Wwh", (M, H), F32, kind="ExternalInput")
    Wwp_d = nc.dram_tensor("Wwp", (H, O), F32, kind="ExternalInput")
    biou_d = nc.dram_tensor("biou", (3 * M,), F32, kind="ExternalInput")
    bf_d = nc.dram_tensor("bf", (M,), F32, kind="ExternalInput")
    bwh_d = nc.dram_tensor("bwh", (H,), F32, kind="ExternalInput")
    out_d = nc.dram_tensor("out", (Bc,), F32, kind="ExternalOutput")

    emb_pairs = emb_d[:].rearrange("(v two) d -> v two d", two=2)

    with tile.TileContext(nc) as tc, ExitStack() as ctx:
        sb = ctx.enter_context(tc.tile_pool(name="sb", bufs=1))
        ps = ctx.enter_context(tc.tile_pool(name="ps", bufs=1, space="PSUM"))

        # ---- loads ----
        idx_t = sb.tile([128, idx_d.shape[1]], I16)
        nc.sync.dma_start(idx_t[:], idx_d[:])
        wioux_t = sb.tile([128, DC, 3 * M], F32)
        nc.sync.dma_start(wioux_t[:], Wioux_d[:].rearrange("(c p) m -> p c m", p=128))
        wiouh_t = sb.tile([128, MC, 2 * M], F32)
        nc.sync.dma_start(wiouh_t[:, :, :M], Wiouh_d[:, 0:M].rearrange("(c p) m -> p c m", p=128))
        nc.sync.dma_start(wiouh_t[:, :, M:], Wiouh_d[:, 2 * M:3 * M].rearrange("(c p) m -> p c m", p=128))
        wfx_t = sb.tile([128, DC, M], F32)
        nc.sync.dma_start(wfx_t[:], Wfx_d[:].rearrange("(c p) m -> p c m", p=128))
        wfh_t = sb.tile([128, MC, M], F32)
        nc.sync.dma_start(wfh_t[:], Wfh_d[:].rearrange("(c p) m -> p c m", p=128))
        wwh_t = sb.tile([128, MC, H], F32)
        nc.sync.dma_start(wwh_t[:], Wwh_d[:].rearrange("(c p) m -> p c m", p=128))
        wwp_t = sb.tile([128, HC, O], F32)
        nc.sync.dma_start(wwp_t[:], Wwp_d[:].rearrange("(c p) m -> p c m", p=128))
        biou_t = sb.tile([128, 3 * MC], F32)
        nc.sync.dma_start(biou_t[:], biou_d[:].rearrange("(c p) -> p c", p=128))
        bf_t = sb.tile([128, MC], F32)
        nc.sync.dma_start(bf_t[:], bf_d[:].rearrange("(c p) -> p c", p=128))
        bwh_t = sb.tile([128, HC], F32)
        nc.sync.dma_start(bwh_t[:], bwh_d[:].rearrange("(c p) -> p c", p=128))

        wsum_t = sb.tile([128, HC], F32)
        for c in range(HC):
            nc.vector.reduce_sum(wsum_t[:, c:c + 1], wwp_t[:, c, :], axis=mybir.AxisListType.X)
        ones_t = sb.tile([128, 1], F32)
        nc.vector.memset(ones_t[:], 1.0)
        zeros_t = sb.tile([128, 256], F32)
        nc.vector.memset(zeros_t[:], 0.0)

        # ---- gather + pooling ----
        # idx column layout: streams [seq0,seq1,seq2,attr0..6], within a stream
        # parity 0 then parity 1; cols per (stream, parity) = CAP/16.
        state = {"col": 0, "q": 0}

        def pool_stream(is_seq, pool_ps, memb_dram, memb_G, slab_base, nsl, out_cols_fn):
            for e in range(2):
                s0 = 0
                while s0 < nsl:
                    ns = min(SPC, nsl - s0)
                    c0 = state["col"]
                    state["col"] += ns * 8
                    g = sb.tile([128, SPC, D], F32, name="g", tag="g", bufs=3)
                    mt = sb.tile([128, SPC, memb_G], F32, name="mt",
                                 tag="mt" + ("s" if is_seq else "a"), bufs=3)
                    so = slab_base + e * nsl + s0
                    nc.sync.dma_start(mt[:, :ns, :], memb_dram[:, so:so + ns, :])
                    nc.gpsimd.dma_gather(
                        out_ap=g[:, :ns, :],
                        in_ap=emb_pairs[:, e, :],
                        idxs_ap=idx_t[:, c0:c0 + ns * 8],
                        num_idxs=ns * 128,
                        num_idxs_reg=ns * 128,
                        elem_size=D,
                        elem_step=2 * D,
                    )
                    for j in range(ns):
                        s = s0 + j
                        first = (e == 0 and s == 0)
                        last = (e == 1 and s == nsl - 1)
                        for c in range(DC):
                            nc.tensor.matmul(
                                out=out_cols_fn(pool_ps, c, s),
                                lhsT=g[:, j, c * 128:(c + 1) * 128],
                                rhs=mt[:, j, :],
                                start=(False if is_seq else first),
                                stop=last,
                                skip_group_check=True,
                            )
                    s0 += ns

        # seq streams first (their results gate the TreeLSTM GEMMs)
        xT3 = sb.tile([128, DC, 3 * NPT], F32)
        for t in range(3):
            pool_ps = ps.tile([128, DC, NPT], F32, name="pool_ps", tag="pool")
            for c in range(DC):  # zero-prelude: clear has_written + zero cols
                nc.tensor.matmul(out=pool_ps[:, c, :], lhsT=zeros_t[:, :128],
                                 rhs=zeros_t[:, :NPT], start=True, stop=False,
                                 skip_group_check=True)

            def seq_cols(pp, c, s):
                base = _seq_base(s, span, WB, Bc)
                return pp[:, c, :].rearrange("p (b n) -> p b n", n=4)[:, base:base + WB, :]

            pool_stream(True, pool_ps, memb_s_d, 4 * WB, t * 2 * SL_S, SL_S, seq_cols)
            nc.vector.tensor_copy(xT3[:, :, t * NPT:(t + 1) * NPT], pool_ps[:])

        attr_sb = sb.tile([128, 7, DC, Bc], F32)
        for k in range(7):
            pool_psa = ps.tile([128, DC, Bc], F32, name="pool_psa", tag="pool")

            def attr_cols(pp, c, s):
                return pp[:, c, :]

            pool_stream(False, pool_psa, memb_a_d, Bc, k * 2 * SL_A, SL_A, attr_cols)
            nc.vector.tensor_copy(attr_sb[:, k], pool_psa[:])

        # ---- TreeLSTM leaves ----
        # col layouts: xT3 per tree: 4b+node; leaves (cL/hL): 3b+j; root (cr): t*Bc+b.
        cL = sb.tile([128, MC, 3 * LB], F32)
        hL = sb.tile([128, MC, 3 * LB], F32)
        for t in range(3):
            xleaf = xT3[:, :, t * NPT:(t + 1) * NPT].rearrange("p c (b n) -> p c b n", n=4)[:, :, :, 0:3]
            for r in range(2):  # mc rounds {0,1},{2,3}
                iou_ps = ps.tile([128, 6, 256], F32, name="iou_ps", tag="psA")
                for i, mc in enumerate([2 * r, 2 * r + 1]):
                    for part in range(3):  # i, o, u
                        for kc in range(DC):
                            nc.tensor.matmul(
                                out=iou_ps[:, part * 2 + i, :LB],
                                lhsT=wioux_t[:, kc, (part * MC + mc) * 128:(part * MC + mc + 1) * 128],
                                rhs=xleaf[:, kc],
                                start=(kc == 0), stop=(kc == DC - 1),
                            )
                ti = sb.tile([128, LB], F32, name="ti", tag="ti")
                tu = sb.tile([128, LB], F32, name="tu", tag="tu")
                to = sb.tile([128, LB], F32, name="to", tag="to")
                for i, mc in enumerate([2 * r, 2 * r + 1]):
                    nc.scalar.activation(ti[:], iou_ps[:, i, :LB], AF.Sigmoid, bias=biou_t[:, mc:mc + 1])
                    nc.scalar.activation(to[:], iou_ps[:, 2 + i, :LB], AF.Sigmoid, bias=biou_t[:, MC + mc:MC + mc + 1])
                    nc.scalar.activation(tu[:], iou_ps[:, 4 + i, :LB], AF.Tanh, bias=biou_t[:, 2 * MC + mc:2 * MC + mc + 1])
                    nc.vector.tensor_mul(cL[:, mc, t * LB:(t + 1) * LB], ti[:], tu[:])
                    nc.scalar.activation(ti[:], cL[:, mc, t * LB:(t + 1) * LB], AF.Tanh)
                    nc.vector.tensor_mul(hL[:, mc, t * LB:(t + 1) * LB], to[:], ti[:])

        # ---- root ----
        hs = sb.tile([128, MC, 3 * Bc], F32)  # cols t*Bc+b
        for t in range(3):
            for c in range(MC):
                hj = hL[:, c, t * LB:(t + 1) * LB].rearrange("p (b j) -> p b j", j=3)
                nc.vector.tensor_add(hs[:, c, t * Bc:(t + 1) * Bc], hj[:, :, 0], hj[:, :, 1])
                nc.vector.tensor_add(hs[:, c, t * Bc:(t + 1) * Bc],
                                     hs[:, c, t * Bc:(t + 1) * Bc], hj[:, :, 2])

        xroot = xT3[:, :, :].rearrange("p c (t b n) -> p c t b n", t=3, n=4)[:, :, :, :, 3]

        # f gates (mc rounds of 2), g = Wfx @ x_root
        f_sb = sb.tile([128, MC, 3 * LB], F32)
        g_ps = ps.tile([128, MC, 256], F32, name="g_ps", tag="psB")
        for mc in range(MC):
            for kc in range(DC):
                nc.tensor.matmul(
                    out=g_ps[:, mc, :3 * Bc],
                    lhsT=wfx_t[:, kc, mc * 128:(mc + 1) * 128],
                    rhs=xroot[:, kc],
                    start=(kc == 0), stop=(kc == DC - 1),
                )
        g_sb = sb.tile([128, MC, 3 * Bc], F32)
        nc.vector.tensor_copy(g_sb[:], g_ps[:, :, :3 * Bc])
        for r in range(2):
            f_ps = ps.tile([128, 2, 3 * PS_T], F32, name="f_ps", tag="psA")
            for i, mc in enumerate([2 * r, 2 * r + 1]):
                for t in range(3):
                    for kc in range(MC):
                        nc.tensor.matmul(
                            out=f_ps[:, i, t * PS_T:t * PS_T + LB],
                            lhsT=wfh_t[:, kc, mc * 128:(mc + 1) * 128],
                            rhs=hL[:, kc, t * LB:(t + 1) * LB],
                            start=(kc == 0), stop=(kc == MC - 1),
                        )
            for i, mc in enumerate([2 * r, 2 * r + 1]):
                nc.vector.tensor_add(
                    f_sb[:, mc, :].rearrange("p (t b j) -> p t b j", t=3, j=3),
                    f_ps[:, i, :].rearrange("p (t x) -> p t x", t=3)[:, :, :LB].rearrange("p t (b j) -> p t b j", j=3),
                    g_sb[:, mc, :].rearrange("p (t b) -> p t b", t=3)[:, :, :, None].to_broadcast([128, 3, Bc, 3]),
                )
                nc.scalar.activation(f_sb[:, mc, :], f_sb[:, mc, :], AF.Sigmoid, bias=bf_t[:, mc:mc + 1])

        # root i,u + c_root
        cr = sb.tile([128, MC, 3 * Bc], F32)
        ri = sb.tile([128, 3 * Bc], F32, name="ri", tag="ti")
        ru = sb.tile([128, 3 * Bc], F32, name="ru", tag="tu")
        for r in range(2):
            riou_ps = ps.tile([128, 4, 256], F32, name="riou_ps", tag="psA")
            for i, mc in enumerate([2 * r, 2 * r + 1]):
                for half, wof in ((0, 0), (1, M)):
                    for kc in range(DC):
                        nc.tensor.matmul(
                            out=riou_ps[:, half * 2 + i, :3 * Bc],
                            lhsT=(wioux_t[:, kc, mc * 128:(mc + 1) * 128] if half == 0
                                  else wioux_t[:, kc, (2 * MC + mc) * 128:(2 * MC + mc + 1) * 128]),
                            rhs=xroot[:, kc],
                            start=(kc == 0), stop=False,
                        )
                    for kc in range(MC):
                        nc.tensor.matmul(
                            out=riou_ps[:, half * 2 + i, :3 * Bc],
                            lhsT=wiouh_t[:, kc, wof + mc * 128:wof + (mc + 1) * 128],
                            rhs=hs[:, kc, :],
                            start=False, stop=(kc == MC - 1),
                        )
            for i, mc in enumerate([2 * r, 2 * r + 1]):
                nc.scalar.activation(ri[:], riou_ps[:, i, :3 * Bc], AF.Sigmoid, bias=biou_t[:, mc:mc + 1])
                nc.scalar.activation(ru[:], riou_ps[:, 2 + i, :3 * Bc], AF.Tanh, bias=biou_t[:, 2 * MC + mc:2 * MC + mc + 1])
                nc.vector.tensor_mul(cr[:, mc, :], ri[:], ru[:])
        for c in range(MC):
            fc_c = sb.tile([128, 3 * LB], F32, name="fc_c", tag="to")
            nc.vector.tensor_mul(fc_c[:], f_sb[:, c, :], cL[:, c, :])
            for j in range(3):
                nc.vector.tensor_add(
                    cr[:, c, :].rearrange("p (t b) -> p t b", t=3),
                    cr[:, c, :].rearrange("p (t b) -> p t b", t=3),
                    fc_c[:].rearrange("p (t b j) -> p t b j", t=3, j=3)[:, :, :, j],
                )

        # ---- similarity ----
        zq = sb.tile([128, DC, 2 * Bc], F32)
        for c in range(MC):
            nc.vector.tensor_mul(
                zq[:, c, :].rearrange("p (r b) -> p r b", r=2),
                cr[:, c, 0:Bc][:, None, :].to_broadcast([128, 2, Bc]),
                cr[:, c, Bc:3 * Bc].rearrange("p (r b) -> p r b", r=2),
            )
        sh_ps = ps.tile([128, HC, 128], F32, name="sh_ps", tag="pool")
        for hc in range(HC):
            for kc in range(MC):
                nc.tensor.matmul(
                    out=sh_ps[:, hc, :2 * Bc],
                    lhsT=wwh_t[:, kc, hc * 128:(hc + 1) * 128],
                    rhs=zq[:, kc, :],
                    start=(kc == 0), stop=(kc == MC - 1),
                )
        sig_sb = sb.tile([128, HC, 2 * Bc], F32)
        for hc in range(HC):
            nc.scalar.activation(sig_sb[:, hc, :], sh_ps[:, hc, :2 * Bc], AF.Sigmoid, bias=bwh_t[:, hc:hc + 1])
        ab_ps = ps.tile([1, 2 * Bc], F32, name="ab_ps", tag="pool")
        for hc in range(HC):
            nc.tensor.matmul(
                out=ab_ps[:, :], lhsT=wsum_t[:, hc:hc + 1], rhs=sig_sb[:, hc, :],
                start=(hc == 0), stop=(hc == HC - 1),
            )
        ab_sb = sb.tile([1, 2 * Bc], F32)
        nc.vector.tensor_copy(ab_sb[:], ab_ps[:1, :])
        dab = sb.tile([1, Bc], F32)
        nc.vector.tensor_sub(dab[:], ab_sb[:1, Bc:2 * Bc], ab_sb[:1, 0:Bc])
        hinge = sb.tile([1, Bc], F32)
        nc.scalar.activation(hinge[:], dab[:], AF.Relu, bias=1.0)

        # ---- triplet losses ----
        dt = sb.tile([128, DC, Bc], F32, name="dt", tag="ti")
        mt2 = sb.tile([128, DC, Bc], F32, name="mt2", tag="tu")
        dots_ps = ps.tile([1, 3, Bc], F32, name="dots_ps", tag="pool")
        for k in range(3):
            nc.vector.tensor_sub(dt[:], attr_sb[:, 1 + 2 * k], attr_sb[:, 2 + 2 * k])
            nc.vector.tensor_mul(mt2[:], attr_sb[:, 0], dt[:])
            for c in range(DC):
                nc.tensor.matmul(
                    out=dots_ps[:1, k, :], lhsT=ones_t[:], rhs=mt2[:, c, :],
                    start=(c == 0), stop=(c == DC - 1),
                )
        loss3 = sb.tile([1, 3, Bc], F32)
        nc.scalar.activation(loss3[:1, :, :], dots_ps[:1, :, :], AF.Relu, bias=1.0, scale=-1.0)
        loss = sb.tile([1, Bc], F32)
        nc.vector.tensor_add(loss[:], loss3[:1, 0, :], loss3[:1, 1, :])
        nc.vector.tensor_add(loss[:], loss[:], loss3[:1, 2, :])

        fin = sb.tile([1, Bc], F32)
        nc.vector.tensor_add(fin[:], loss[:], hinge[:])
        nc.sync.dma_start(out_d[None, :], fin[:1, :])

    nc.compile()
    return nc


_PROG_CACHE = {}


def _get_program(*args):
    if args not in _PROG_CACHE:
        _PROG_CACHE[args] = build_program(*args)
    return _PROG_CACHE[args]


def _wrap_idx(flat):
    """[n] -> [128, n/16] int16 wrapped (flat i = s*16 + p), replicated x8."""
    w = flat.reshape(-1, 16).T
    return np.tile(w, (8, 1)).astype(np.int16)


def _prep_core_inputs(inputs, ci, Bc, L, LQ):
    sl = slice(ci * Bc, (ci + 1) * Bc)
    CAP_A, CAP_S = _cap(Bc * L), _cap(Bc * LQ)
    SL_A, SL_S = CAP_A // 128, CAP_S // 128
    span, WB = _seq_window(Bc, LQ)
    npn = LQ // 4

    idx_cols = []
    memb_s = np.zeros((128, 3 * 2 * SL_S, 4 * WB), np.float32)
    memb_a = np.zeros((128, 7 * 2 * SL_A, Bc), np.float32)

    def add_stream(tokens, cap, memb, slab_base, col_fn, w):
        nsl = cap // 128
        for e in range(2):
            pos = np.nonzero((tokens % 2) == e)[0]
            assert len(pos) <= cap, f"parity capacity exceeded: {len(pos)} > {cap}"
            pid = (tokens[pos] // 2).astype(np.int16)
            pad = np.zeros(cap - len(pos), np.int16)
            idx_cols.append(_wrap_idx(np.concatenate([pid, pad])))
            i = np.arange(len(pos))
            s, p = i // 128, i % 128
            memb[p, slab_base + e * nsl + s, col_fn(pos, s)] = w

    for t, key in enumerate(SEQ_KEYS):
        toks = np.asarray(inputs[key][sl], dtype=np.int64).reshape(-1)

        def col_fn(pos, s):
            b, node = pos // LQ, (pos % LQ) // npn
            base = np.clip(span * s - (WB - span) // 2, 0, Bc - WB)
            db = b - base
            assert (db >= 0).all() and (db < WB).all(), "seq window violated"
            return db * 4 + node

        add_stream(toks, CAP_S, memb_s, t * 2 * SL_S, col_fn, 1.0 / npn)

    for k, key in enumerate(ATTR_KEYS):
        toks = np.asarray(inputs[key][sl], dtype=np.int64).reshape(-1)
        add_stream(toks, CAP_A, memb_a, k * 2 * SL_A,
                   lambda pos, s: pos // L, 1.0 / L)

    f32 = lambda k: np.ascontiguousarray(np.asarray(inputs[k], dtype=np.float32))
    return {
        "emb": f32("emb"),
        "idx": np.ascontiguousarray(np.concatenate(idx_cols, axis=1)),
        "memb_s": memb_s,
        "memb_a": memb_a,
        "Wioux": f32("Wioux"), "Wiouh": f32("Wiouh"),
        "Wfx": f32("Wfx"), "Wfh": f32("Wfh"),
        "Wwh": f32("Wwh"), "Wwp": f32("Wwp"),
        "biou": f32("bioux") + f32("biouh"),
        "bf": f32("bfx") + f32("bfh"),
        "bwh": f32("bwh"),
    }


def kernel(**inputs) -> np.ndarray:
    Bc = B // NC_CORES
    nc = _get_program(Bc, L, LQ, V, D, M, H, O)
    in_maps = [_prep_core_inputs(inputs, ci, Bc, L, LQ) for ci in range(NC_CORES)]
    res = run_bass_kernel_spmd(nc, in_maps, core_ids=list(range(NC_CORES)))
    return np.concatenate([res.results[ci]["out"] for ci in range(NC_CORES)])
